# revision 16
# baseline (speedup 1.0000x reference)
"""Trainium2 Bass kernel for nn_AttentionLayer (B=4, S=4096, D=128, fp32).

Sharding: batch (4) x query-half (2) across 8 NeuronCores. Each core computes
single-head attention for one batch element over a 2048-query slice with full
4096-key context.

Per-core dataflow (all on-chip after the x^T load):
  K^T[e,t] = WkT.T @ x^T          (PE, f32r, N=512)
  Q^T[e,s] = WqT.T @ x^T[:,qcols] (PE, f32r, N=512)
  V[t,e]   = x^T-chunk.T @ WvT    (PE, 32 chunks, N=128)
  alpha[t] = x^T-chunk.T @ wtl    (PE, N=1; wtl = scale * Wk.T @ bq)
  scoresT[t-chunk, s] = K^T-chunk.T @ Q^T   (PE -> PSUM)
  expT = exp(scale*scoresT + alpha)         (ACT, PSUM -> SBUF)
  outT[e,s]  += V-chunk.T @ expT            (PE, PSUM accumulate)
  denom[*,s] += ones.T @ expT               (PE, PSUM accumulate, replicated)
  out = (outT * 1/denom + bv).T             (DVE + PE transpose), DMA out.

Bias algebra: the query-side bias terms (q0.bk, bq.bk) are constant in t and
cancel in softmax; the key-side term bq.k0[t] is folded into the exp bias via
alpha = x^T.T @ (scale * Wk.T @ bq). bk drops out entirely; bv is added at the
end (sum of attention weights is 1 after normalization).
"""

import sys

import numpy as np

for _p in ("/opt/trn_rl_repo", "/opt/pypackages"):
    if _p not in sys.path:
        sys.path.append(_p)

B, S, D = 4, 4096, 128
N_CORES = 8
SQ = S // 2  # queries per core
SCALE = 1.0 / float(np.sqrt(D))


def build_attention_bass(s=S, sq=SQ, sw=1024):
    """Build the single-core SPMD Bass program.

    s: key/context length; sq: queries handled by the core; sw: query-pass
    width (PSUM budget: 2*sw*4B of score buffers + sw*4B out + sw*4B denom
    per partition must fit 16KB -> sw=1024 uses exactly 8 banks).
    """
    import concourse.bass as bass
    import concourse.mybir as mybir
    import concourse.tile as tile
    from concourse import bacc
    from concourse.masks import make_identity
    from contextlib import ExitStack

    f32 = mybir.dt.float32
    f32r = mybir.dt.float32r
    FT = mybir.ActivationFunctionType

    tch = s // 128          # key chunks
    n_pass = sq // sw       # query passes
    nw = min(512, sw)       # matmul N width
    jn = sw // nw           # matmuls per pass-width

    def chunks(total, w=512):
        for st in range(0, total, w):
            yield st, min(w, total - st)

    nc = bacc.Bacc("TRN2", target_bir_lowering=False, debug=False)

    xT = nc.dram_tensor("xT", [D, s], f32r, kind="ExternalInput").ap()
    xTq = nc.dram_tensor("xTq", [D, sq], f32r, kind="ExternalInput").ap()
    wqT = nc.dram_tensor("wqT", [D, D], f32r, kind="ExternalInput").ap()
    wkT = nc.dram_tensor("wkT", [D, D], f32r, kind="ExternalInput").ap()
    wvT = nc.dram_tensor("wvT", [D, D], f32r, kind="ExternalInput").ap()
    wtl = nc.dram_tensor("wtl", [D, 2], f32r, kind="ExternalInput").ap()
    bv = nc.dram_tensor("bv", [D, 1], f32, kind="ExternalInput").ap()
    out_d = nc.dram_tensor("out", [sq, D], f32, kind="ExternalOutput").ap()

    with tile.TileContext(nc) as tc, ExitStack() as ctx:
        const = ctx.enter_context(tc.tile_pool(name="const", bufs=1))
        big = ctx.enter_context(tc.tile_pool(name="big", bufs=1))
        exp_pool = ctx.enter_context(tc.tile_pool(name="exp", bufs=3))
        epi = ctx.enter_context(tc.tile_pool(name="epi", bufs=2))
        outp = ctx.enter_context(tc.tile_pool(name="outp", bufs=3))

        # ---- constants / weights
        wq_sb = const.tile([D, D], f32r, tag="wq")
        wk_sb = const.tile([D, D], f32r, tag="wk")
        wv_sb = const.tile([D, D], f32r, tag="wv")
        wtl_sb = const.tile([D, 2], f32r, tag="wtl")
        bv_sb = const.tile([D, 1], f32, tag="bv")
        ones_sb = const.tile([128, 128], f32r, tag="ones")
        ident_sb = const.tile([128, 128], f32, tag="ident")
        nc.sync.dma_start(wq_sb[:], wqT)
        nc.sync.dma_start(wk_sb[:], wkT)
        nc.sync.dma_start(wv_sb[:], wvT)
        nc.sync.dma_start(wtl_sb[:], wtl)
        nc.sync.dma_start(bv_sb[:], bv)
        make_identity(nc, ident_sb[:])
        # f32r memset is not a legal ISA instruction; synthesize ones on ACT
        nc.scalar.activation(ones_sb[:], ident_sb[:],
                             FT.Identity, bias=1.0, scale=0.0)

        # ---- load x^T (split DMAs so chunks land independently)
        xT_sb = big.tile([D, s], f32r, tag="xT")
        for st, w in chunks(s):
            nc.sync.dma_start(xT_sb[:, st:st + w], xT[:, st:st + w])
        xTq_sb = big.tile([D, sq], f32r, tag="xTq")
        for st, w in chunks(sq):
            nc.sync.dma_start(xTq_sb[:, st:st + w], xTq[:, st:st + w])

        kt_sb = big.tile([D, s], f32r, tag="kt")
        qt_sb = big.tile([D, sq], f32r, tag="qt")
        v_sb = big.tile([128, s], f32r, tag="v")
        alpha_sb = const.tile([128, 2 * tch], f32, tag="alpha")

        # ---- projections
        with tc.tile_pool(name="qkps", bufs=3, space="PSUM") as qkps, \
             tc.tile_pool(name="vps", bufs=3, space="PSUM") as vps, \
             tc.tile_pool(name="aps", bufs=1, space="PSUM") as apsp:
            for j, (st, w) in enumerate(chunks(s)):
                kp = qkps.tile([128, 512], f32, tag="kp")
                nc.tensor.matmul(kp[:, :w], wk_sb[:],
                                 xT_sb[:, st:st + w])
                if j % 2 == 0:
                    nc.scalar.copy(kt_sb[:, st:st + w], kp[:, :w])
                else:
                    nc.vector.tensor_copy(kt_sb[:, st:st + w], kp[:, :w])
            for j, (st, w) in enumerate(chunks(sq)):
                qp = qkps.tile([128, 512], f32, tag="kp")
                nc.tensor.matmul(qp[:, :w], wq_sb[:],
                                 xTq_sb[:, st:st + w])
                if j % 2 == 0:
                    nc.scalar.copy(qt_sb[:, st:st + w], qp[:, :w])
                else:
                    nc.vector.tensor_copy(qt_sb[:, st:st + w], qp[:, :w])
            ap_ps = apsp.tile([128, 2 * tch], f32, tag="aps")
            for c in range(tch):
                vp = vps.tile([128, 128], f32, tag="vp")
                xc = xT_sb[:, c * 128:(c + 1) * 128]
                nc.tensor.matmul(vp[:], xc, wv_sb[:])
                nc.tensor.matmul(ap_ps[:, 2 * c:2 * c + 2], xc, wtl_sb[:])
                if c % 2 == 0:
                    nc.scalar.copy(v_sb[:, c * 128:(c + 1) * 128], vp[:])
                else:
                    nc.vector.tensor_copy(v_sb[:, c * 128:(c + 1) * 128], vp[:])
            nc.vector.tensor_copy(alpha_sb[:], ap_ps[:])

        # ---- attention passes
        with tc.tile_pool(name="scps", bufs=2, space="PSUM") as scps, \
             tc.tile_pool(name="accps", bufs=1, space="PSUM") as accps:
            for p in range(n_pass):
                acc_o = accps.tile([128, sw], f32, tag="acco")
                acc_d = accps.tile([128, sw], f32, tag="accd")
                for c in range(tch):
                    sc = scps.tile([128, sw], f32, tag="sc")
                    kc = kt_sb[:, c * 128:(c + 1) * 128]
                    for j in range(jn):
                        nc.tensor.matmul(
                            sc[:, j * nw:(j + 1) * nw], kc,
                            qt_sb[:, p * sw + j * nw: p * sw + (j + 1) * nw])
                    et = exp_pool.tile([128, sw], f32r, tag="et")
                    nc.scalar.activation(et[:], sc[:], FT.Exp,
                                         bias=alpha_sb[:, 2 * c:2 * c + 1],
                                         scale=SCALE)
                    vc = v_sb[:, c * 128:(c + 1) * 128]
                    for j in range(jn):
                        ej = et[:, j * nw:(j + 1) * nw]
                        nc.tensor.matmul(acc_o[:, j * nw:(j + 1) * nw], vc, ej,
                                         start=(c == 0), stop=(c == tch - 1))
                        nc.tensor.matmul(acc_d[:, j * nw:(j + 1) * nw],
                                         ones_sb[:], ej,
                                         start=(c == 0), stop=(c == tch - 1))
                # normalize in [e, s] layout, add bv, transpose out
                recip = epi.tile([128, sw], f32, tag="recip")
                nc.vector.reciprocal_approx_fast(recip[:], acc_d[:])
                norm = epi.tile([128, sw], f32, tag="norm")
                nc.vector.tensor_mul(norm[:], acc_o[:], recip[:])
                norm2 = epi.tile([128, sw], f32, tag="norm2")
                nc.vector.tensor_scalar_add(norm2[:], norm[:], bv_sb[:])
                for k in range(sw // 128):
                    tp = scps.tile([128, 128], f32, tag="sc")
                    nc.tensor.transpose(tp[:], norm2[:, k * 128:(k + 1) * 128],
                                        ident_sb[:])
                    ot = outp.tile([128, 128], f32, tag="ot")
                    nc.vector.tensor_copy(ot[:], tp[:])
                    nc.sync.dma_start(
                        out_d[p * sw + k * 128: p * sw + (k + 1) * 128, :],
                        ot[:])
    nc.compile()
    return nc


def make_in_maps(x, Wq, bq, Wk, Wv, bv, s=S, sq=SQ, n_cores=N_CORES):
    """Per-core input dict list. Core c -> batch c//(cores per batch),
    query slice (c % per_b) * sq."""
    x = np.asarray(x, np.float32)
    nb = x.shape[0]
    per_b = n_cores // nb
    wq_t = np.ascontiguousarray(np.asarray(Wq, np.float32).T)
    wk_t = np.ascontiguousarray(np.asarray(Wk, np.float32).T)
    wv_t = np.ascontiguousarray(np.asarray(Wv, np.float32).T)
    wtl = np.repeat((SCALE * (wk_t @ np.asarray(bq, np.float32)))
                    .reshape(D, 1), 2, axis=1)
    bvc = np.asarray(bv, np.float32).reshape(D, 1)
    maps = []
    for c in range(n_cores):
        b, h = c // per_b, c % per_b
        xt = np.ascontiguousarray(x[b].T)
        maps.append({
            "xT": xt,
            "xTq": np.ascontiguousarray(xt[:, h * sq:(h + 1) * sq]),
            "wqT": wq_t, "wkT": wk_t, "wvT": wv_t,
            "wtl": np.ascontiguousarray(wtl, dtype=np.float32),
            "bv": np.ascontiguousarray(bvc, dtype=np.float32),
        })
    return maps


_NC_CACHE = {}


def _get_nc():
    if "nc" not in _NC_CACHE:
        _NC_CACHE["nc"] = build_attention_bass()
    return _NC_CACHE["nc"]


def run_on_hw(inputs, trace=False, **kw):
    from concourse.bass_utils import run_bass_kernel_spmd
    nc = _get_nc()
    maps = make_in_maps(inputs["x"], inputs["Wq"], inputs["bq"], inputs["Wk"],
                        inputs["Wv"], inputs["bv"])
    res = run_bass_kernel_spmd(nc, maps, core_ids=list(range(N_CORES)),
                               trace=trace, **kw)
    nb = np.asarray(inputs["x"]).shape[0]
    per_b = N_CORES // nb
    out = np.empty((nb, S * D), np.float32)
    for c in range(N_CORES):
        b, h = c // per_b, c % per_b
        out[b, h * SQ * D:(h + 1) * SQ * D] = \
            np.asarray(res.results[c]["out"]).reshape(-1)
    return out, res


def kernel(**inputs):
    out, _ = run_on_hw(inputs, trace=False)
    return out


# revision 18
# speedup vs baseline: 1.1462x; 1.1462x over previous
"""Trainium2 Bass kernel for nn_AttentionLayer (B=4, S=4096, D=128, fp32).

Sharding: batch (4) x query-half (2) across 8 NeuronCores. Each core computes
single-head attention for one batch element over a 2048-query slice with full
4096-key context.

Per-core dataflow (all on-chip after the x^T load):
  K^T[e,t] = WkT.T @ x^T          (PE, f32r, N=512)
  Q^T[e,s] = WqT.T @ x^T[:,qcols] (PE, f32r, N=512)
  V[t,e]   = x^T-chunk.T @ WvT    (PE, 32 chunks, N=128)
  alpha[t] = x^T-chunk.T @ wtl    (PE, N=1; wtl = scale * Wk.T @ bq)
  scoresT[t-chunk, s] = K^T-chunk.T @ Q^T   (PE -> PSUM)
  expT = exp(scale*scoresT + alpha)         (ACT, PSUM -> SBUF)
  outT[e,s]  += V-chunk.T @ expT            (PE, PSUM accumulate)
  denom[*,s] += ones.T @ expT               (PE, PSUM accumulate, replicated)
  out = (outT * 1/denom + bv).T             (DVE + PE transpose), DMA out.

Bias algebra: the query-side bias terms (q0.bk, bq.bk) are constant in t and
cancel in softmax; the key-side term bq.k0[t] is folded into the exp bias via
alpha = x^T.T @ (scale * Wk.T @ bq). bk drops out entirely; bv is added at the
end (sum of attention weights is 1 after normalization).
"""

import sys

import numpy as np

for _p in ("/opt/trn_rl_repo", "/opt/pypackages"):
    if _p not in sys.path:
        sys.path.append(_p)

B, S, D = 4, 4096, 128
N_CORES = 8
SQ = S // 2  # queries per core
SCALE = 1.0 / float(np.sqrt(D))


def build_attention_bass(s=S, sq=SQ, sw=1024):
    """Build the single-core SPMD Bass program.

    s: key/context length; sq: queries handled by the core; sw: query-pass
    width (PSUM budget: 2*sw*4B of score buffers + sw*4B out + sw*4B denom
    per partition must fit 16KB -> sw=1024 uses exactly 8 banks).
    """
    import concourse.bass as bass
    import concourse.mybir as mybir
    import concourse.tile as tile
    from concourse import bacc
    from concourse.masks import make_identity
    from contextlib import ExitStack

    f32 = mybir.dt.float32
    f32r = mybir.dt.float32r
    FT = mybir.ActivationFunctionType

    tch = s // 128          # key chunks
    n_pass = sq // sw       # query passes
    nw = min(512, sw)       # matmul N width
    jn = sw // nw           # matmuls per pass-width

    def chunks(total, w=512):
        for st in range(0, total, w):
            yield st, min(w, total - st)

    nc = bacc.Bacc("TRN2", target_bir_lowering=False, debug=False)

    xT = nc.dram_tensor("xT", [D, s], f32r, kind="ExternalInput").ap()
    xTq = nc.dram_tensor("xTq", [D, sq], f32r, kind="ExternalInput").ap()
    wqT = nc.dram_tensor("wqT", [D, D], f32r, kind="ExternalInput").ap()
    wkT = nc.dram_tensor("wkT", [D, D], f32r, kind="ExternalInput").ap()
    wvT = nc.dram_tensor("wvT", [D, D], f32r, kind="ExternalInput").ap()
    wtl = nc.dram_tensor("wtl", [D, 2], f32r, kind="ExternalInput").ap()
    bv = nc.dram_tensor("bv", [D, 1], f32, kind="ExternalInput").ap()
    out_d = nc.dram_tensor("out", [sq, D], f32, kind="ExternalOutput").ap()

    with tile.TileContext(nc) as tc, ExitStack() as ctx:
        const = ctx.enter_context(tc.tile_pool(name="const", bufs=1))
        big = ctx.enter_context(tc.tile_pool(name="big", bufs=1))
        exp_pool = ctx.enter_context(tc.tile_pool(name="exp", bufs=3))
        epi = ctx.enter_context(tc.tile_pool(name="epi", bufs=2))
        outp = ctx.enter_context(tc.tile_pool(name="outp", bufs=3))

        # ---- constants / weights
        wq_sb = const.tile([D, D], f32r, tag="wq")
        wk_sb = const.tile([D, D], f32r, tag="wk")
        wv_sb = const.tile([D, D], f32r, tag="wv")
        wtl_sb = const.tile([D, 2], f32r, tag="wtl")
        bv_sb = const.tile([D, 1], f32, tag="bv")
        ones_sb = const.tile([128, 128], f32r, tag="ones")
        ident_sb = const.tile([128, 128], f32, tag="ident")
        nc.sync.dma_start(wq_sb[:], wqT)
        nc.sync.dma_start(wk_sb[:], wkT)
        nc.sync.dma_start(wv_sb[:], wvT)
        nc.sync.dma_start(wtl_sb[:], wtl)
        nc.sync.dma_start(bv_sb[:], bv)
        make_identity(nc, ident_sb[:])
        # f32r memset is not a legal ISA instruction; synthesize ones on ACT
        nc.scalar.activation(ones_sb[:], ident_sb[:],
                             FT.Identity, bias=1.0, scale=0.0)

        # ---- load x^T (split DMAs so chunks land independently)
        xT_sb = big.tile([D, s], f32r, tag="xT")
        for st, w in chunks(s):
            nc.sync.dma_start(xT_sb[:, st:st + w], xT[:, st:st + w])
        xTq_sb = big.tile([D, sq], f32r, tag="xTq")
        for st, w in chunks(sq):
            nc.sync.dma_start(xTq_sb[:, st:st + w], xTq[:, st:st + w])

        kt_sb = big.tile([D, s], f32r, tag="kt")
        qt_sb = big.tile([D, sq], f32r, tag="qt")
        v_sb = big.tile([128, s], f32r, tag="v")
        alpha_sb = const.tile([128, 2 * tch], f32, tag="alpha")

        # ---- projections
        with tc.tile_pool(name="qkps", bufs=3, space="PSUM") as qkps, \
             tc.tile_pool(name="vps", bufs=3, space="PSUM") as vps, \
             tc.tile_pool(name="aps", bufs=1, space="PSUM") as apsp:
            for j, (st, w) in enumerate(chunks(s)):
                kp = qkps.tile([128, 512], f32, tag="kp")
                nc.tensor.matmul(kp[:, :w], wk_sb[:],
                                 xT_sb[:, st:st + w])
                nc.vector.tensor_copy(kt_sb[:, st:st + w], kp[:, :w])
            for j, (st, w) in enumerate(chunks(sq)):
                qp = qkps.tile([128, 512], f32, tag="kp")
                nc.tensor.matmul(qp[:, :w], wq_sb[:],
                                 xTq_sb[:, st:st + w])
                nc.vector.tensor_copy(qt_sb[:, st:st + w], qp[:, :w])
            ap_ps = apsp.tile([128, 2 * tch], f32, tag="aps")
            for c in range(tch):
                vp = vps.tile([128, 128], f32, tag="vp")
                xc = xT_sb[:, c * 128:(c + 1) * 128]
                nc.tensor.matmul(vp[:], xc, wv_sb[:])
                nc.tensor.matmul(ap_ps[:, 2 * c:2 * c + 2], xc, wtl_sb[:])
                nc.vector.tensor_copy(v_sb[:, c * 128:(c + 1) * 128], vp[:])
            nc.vector.tensor_copy(alpha_sb[:], ap_ps[:])

        # ---- attention passes
        with tc.tile_pool(name="scps", bufs=2, space="PSUM") as scps, \
             tc.tile_pool(name="accps", bufs=1, space="PSUM") as accps:
            for p in range(n_pass):
                acc_o = accps.tile([128, sw], f32, tag="acco")
                acc_d = accps.tile([128, sw], f32, tag="accd")

                def emit_scores_exp(c, p=p):
                    """scores chunk c -> PSUM, then exp -> SBUF (f32r)."""
                    sc = scps.tile([128, sw], f32, tag="sc")
                    kc = kt_sb[:, c * 128:(c + 1) * 128]
                    for j in range(jn):
                        nc.tensor.matmul(
                            sc[:, j * nw:(j + 1) * nw], kc,
                            qt_sb[:, p * sw + j * nw: p * sw + (j + 1) * nw])
                    et = exp_pool.tile([128, sw], f32r, tag="et")
                    nc.scalar.activation(et[:], sc[:], FT.Exp,
                                         bias=alpha_sb[:, 2 * c:2 * c + 1],
                                         scale=SCALE)
                    return et

                # software pipeline: scores/exp run one chunk ahead of the
                # accumulating matmuls so the in-order PE never waits on ACT
                et_next = emit_scores_exp(0)
                for c in range(tch):
                    et = et_next
                    if c + 1 < tch:
                        et_next = emit_scores_exp(c + 1)
                    vc = v_sb[:, c * 128:(c + 1) * 128]
                    for j in range(jn):
                        ej = et[:, j * nw:(j + 1) * nw]
                        nc.tensor.matmul(acc_o[:, j * nw:(j + 1) * nw], vc, ej,
                                         start=(c == 0), stop=(c == tch - 1))
                        nc.tensor.matmul(acc_d[:, j * nw:(j + 1) * nw],
                                         ones_sb[:], ej,
                                         start=(c == 0), stop=(c == tch - 1))
                # normalize in [e, s] layout, add bv, transpose out
                recip = epi.tile([128, sw], f32, tag="recip")
                nc.vector.reciprocal_approx_fast(recip[:], acc_d[:])
                norm = epi.tile([128, sw], f32, tag="norm")
                nc.vector.tensor_mul(norm[:], acc_o[:], recip[:])
                norm2 = epi.tile([128, sw], f32, tag="norm2")
                nc.vector.tensor_scalar_add(norm2[:], norm[:], bv_sb[:])
                for k in range(sw // 128):
                    tp = scps.tile([128, 128], f32, tag="sc")
                    nc.tensor.transpose(tp[:], norm2[:, k * 128:(k + 1) * 128],
                                        ident_sb[:])
                    ot = outp.tile([128, 128], f32, tag="ot")
                    nc.vector.tensor_copy(ot[:], tp[:])
                    nc.sync.dma_start(
                        out_d[p * sw + k * 128: p * sw + (k + 1) * 128, :],
                        ot[:])
    nc.compile()
    return nc


def make_in_maps(x, Wq, bq, Wk, Wv, bv, s=S, sq=SQ, n_cores=N_CORES):
    """Per-core input dict list. Core c -> batch c//(cores per batch),
    query slice (c % per_b) * sq."""
    x = np.asarray(x, np.float32)
    nb = x.shape[0]
    per_b = n_cores // nb
    wq_t = np.ascontiguousarray(np.asarray(Wq, np.float32).T)
    wk_t = np.ascontiguousarray(np.asarray(Wk, np.float32).T)
    wv_t = np.ascontiguousarray(np.asarray(Wv, np.float32).T)
    wtl = np.repeat((SCALE * (wk_t @ np.asarray(bq, np.float32)))
                    .reshape(D, 1), 2, axis=1)
    bvc = np.asarray(bv, np.float32).reshape(D, 1)
    maps = []
    for c in range(n_cores):
        b, h = c // per_b, c % per_b
        xt = np.ascontiguousarray(x[b].T)
        maps.append({
            "xT": xt,
            "xTq": np.ascontiguousarray(xt[:, h * sq:(h + 1) * sq]),
            "wqT": wq_t, "wkT": wk_t, "wvT": wv_t,
            "wtl": np.ascontiguousarray(wtl, dtype=np.float32),
            "bv": np.ascontiguousarray(bvc, dtype=np.float32),
        })
    return maps


_NC_CACHE = {}


def _get_nc():
    if "nc" not in _NC_CACHE:
        _NC_CACHE["nc"] = build_attention_bass()
    return _NC_CACHE["nc"]


def run_on_hw(inputs, trace=False, **kw):
    from concourse.bass_utils import run_bass_kernel_spmd
    nc = _get_nc()
    maps = make_in_maps(inputs["x"], inputs["Wq"], inputs["bq"], inputs["Wk"],
                        inputs["Wv"], inputs["bv"])
    res = run_bass_kernel_spmd(nc, maps, core_ids=list(range(N_CORES)),
                               trace=trace, **kw)
    nb = np.asarray(inputs["x"]).shape[0]
    per_b = N_CORES // nb
    out = np.empty((nb, S * D), np.float32)
    for c in range(N_CORES):
        b, h = c // per_b, c % per_b
        out[b, h * SQ * D:(h + 1) * SQ * D] = \
            np.asarray(res.results[c]["out"]).reshape(-1)
    return out, res


def kernel(**inputs):
    out, _ = run_on_hw(inputs, trace=False)
    return out


# revision 20
# speedup vs baseline: 1.1732x; 1.0235x over previous
"""Trainium2 Bass kernel for nn_AttentionLayer (B=4, S=4096, D=128, fp32).

Sharding: batch (4) x query-half (2) across 8 NeuronCores. Each core computes
single-head attention for one batch element over a 2048-query slice with full
4096-key context.

Per-core dataflow (all on-chip after the x^T load):
  K^T[e,t] = WkT.T @ x^T          (PE, f32r, N=512)
  Q^T[e,s] = WqT.T @ x^T[:,qcols] (PE, f32r, N=512)
  V[t,e]   = x^T-chunk.T @ WvT    (PE, 32 chunks, N=128)
  alpha[t] = x^T-chunk.T @ wtl    (PE, N=1; wtl = scale * Wk.T @ bq)
  scoresT[t-chunk, s] = K^T-chunk.T @ Q^T   (PE -> PSUM)
  expT = exp(scale*scoresT + alpha)         (ACT, PSUM -> SBUF)
  outT[e,s]  += V-chunk.T @ expT            (PE, PSUM accumulate)
  denom[*,s] += ones.T @ expT               (PE, PSUM accumulate, replicated)
  out = (outT * 1/denom + bv).T             (DVE + PE transpose), DMA out.

Bias algebra: the query-side bias terms (q0.bk, bq.bk) are constant in t and
cancel in softmax; the key-side term bq.k0[t] is folded into the exp bias via
alpha = x^T.T @ (scale * Wk.T @ bq). bk drops out entirely; bv is added at the
end (sum of attention weights is 1 after normalization).
"""

import sys

import numpy as np

for _p in ("/opt/trn_rl_repo", "/opt/pypackages"):
    if _p not in sys.path:
        sys.path.append(_p)

B, S, D = 4, 4096, 128
N_CORES = 8
SQ = S // 2  # queries per core
SCALE = 1.0 / float(np.sqrt(D))


def build_attention_bass(s=S, sq=SQ, sw=1024):
    """Build the single-core SPMD Bass program.

    s: key/context length; sq: queries handled by the core; sw: query-pass
    width (PSUM budget: 2*sw*4B of score buffers + sw*4B out + sw*4B denom
    per partition must fit 16KB -> sw=1024 uses exactly 8 banks).
    """
    import concourse.bass as bass
    import concourse.mybir as mybir
    import concourse.tile as tile
    from concourse import bacc
    from concourse.masks import make_identity
    from contextlib import ExitStack

    f32 = mybir.dt.float32
    f32r = mybir.dt.float32r
    FT = mybir.ActivationFunctionType

    tch = s // 128          # key chunks
    n_pass = sq // sw       # query passes
    nw = min(512, sw)       # matmul N width
    jn = sw // nw           # matmuls per pass-width

    def chunks(total, w=512):
        for st in range(0, total, w):
            yield st, min(w, total - st)

    nc = bacc.Bacc("TRN2", target_bir_lowering=False, debug=False)

    xT = nc.dram_tensor("xT", [D, s], f32r, kind="ExternalInput").ap()
    xTq = nc.dram_tensor("xTq", [D, sq], f32r, kind="ExternalInput").ap()
    wqT = nc.dram_tensor("wqT", [D, D], f32r, kind="ExternalInput").ap()
    wkT = nc.dram_tensor("wkT", [D, D], f32r, kind="ExternalInput").ap()
    wvT = nc.dram_tensor("wvT", [D, D], f32r, kind="ExternalInput").ap()
    wtl = nc.dram_tensor("wtl", [D, 2], f32r, kind="ExternalInput").ap()
    bv = nc.dram_tensor("bv", [D, 1], f32, kind="ExternalInput").ap()
    out_d = nc.dram_tensor("out", [sq, D], f32, kind="ExternalOutput").ap()

    with tile.TileContext(nc) as tc, ExitStack() as ctx:
        const = ctx.enter_context(tc.tile_pool(name="const", bufs=1))
        big = ctx.enter_context(tc.tile_pool(name="big", bufs=1))
        exp_pool = ctx.enter_context(tc.tile_pool(name="exp", bufs=3))
        epi = ctx.enter_context(tc.tile_pool(name="epi", bufs=2))
        outp = ctx.enter_context(tc.tile_pool(name="outp", bufs=3))

        # ---- constants / weights
        wq_sb = const.tile([D, D], f32r, tag="wq")
        wk_sb = const.tile([D, D], f32r, tag="wk")
        wv_sb = const.tile([D, D], f32r, tag="wv")
        wtl_sb = const.tile([D, 2], f32r, tag="wtl")
        bv_sb = const.tile([D, 1], f32, tag="bv")
        ones_sb = const.tile([128, 128], f32r, tag="ones")
        ident_sb = const.tile([128, 128], f32, tag="ident")
        nc.sync.dma_start(wq_sb[:], wqT)
        nc.sync.dma_start(wk_sb[:], wkT)
        nc.sync.dma_start(wv_sb[:], wvT)
        nc.sync.dma_start(wtl_sb[:], wtl)
        nc.sync.dma_start(bv_sb[:], bv)
        make_identity(nc, ident_sb[:])
        # f32r memset is not a legal ISA instruction; synthesize ones on ACT
        nc.scalar.activation(ones_sb[:], ident_sb[:],
                             FT.Identity, bias=1.0, scale=0.0)

        # ---- load x^T (split DMAs so chunks land independently)
        xT_sb = big.tile([D, s], f32r, tag="xT")
        for st, w in chunks(s):
            nc.sync.dma_start(xT_sb[:, st:st + w], xT[:, st:st + w])
        xTq_sb = big.tile([D, sq], f32r, tag="xTq")
        for st, w in chunks(sq):
            nc.sync.dma_start(xTq_sb[:, st:st + w], xTq[:, st:st + w])

        kt_sb = big.tile([D, s], f32r, tag="kt")
        qt_sb = big.tile([D, sq], f32r, tag="qt")
        v_sb = big.tile([128, s], f32r, tag="v")
        alpha_sb = const.tile([128, 2 * tch], f32, tag="alpha")

        # ---- projections
        with tc.tile_pool(name="qkps", bufs=3, space="PSUM") as qkps, \
             tc.tile_pool(name="vps", bufs=4, space="PSUM") as vps, \
             tc.tile_pool(name="aps", bufs=1, space="PSUM") as apsp:
            for j, (st, w) in enumerate(chunks(s)):
                kp = qkps.tile([128, 512], f32, tag="kp")
                nc.tensor.matmul(kp[:, :w], wk_sb[:],
                                 xT_sb[:, st:st + w])
                nc.vector.tensor_copy(kt_sb[:, st:st + w], kp[:, :w])
            for j, (st, w) in enumerate(chunks(sq)):
                qp = qkps.tile([128, 512], f32, tag="kp")
                nc.tensor.matmul(qp[:, :w], wq_sb[:],
                                 xTq_sb[:, st:st + w])
                nc.vector.tensor_copy(qt_sb[:, st:st + w], qp[:, :w])
            ap_ps = apsp.tile([128, 2 * tch], f32, tag="aps")
            for c in range(tch):
                vp = vps.tile([128, 128], f32, tag="vp")
                xc = xT_sb[:, c * 128:(c + 1) * 128]
                nc.tensor.matmul(vp[:], xc, wv_sb[:])
                nc.tensor.matmul(ap_ps[:, 2 * c:2 * c + 2], xc, wtl_sb[:])
                nc.vector.tensor_copy(v_sb[:, c * 128:(c + 1) * 128], vp[:])
            nc.vector.tensor_copy(alpha_sb[:], ap_ps[:])

        # ---- attention passes
        with tc.tile_pool(name="scps", bufs=2, space="PSUM") as scps, \
             tc.tile_pool(name="accps", bufs=1, space="PSUM") as accps:
            for p in range(n_pass):
                acc_o = accps.tile([128, sw], f32, tag="acco")
                acc_d = accps.tile([128, sw], f32, tag="accd")

                def emit_scores_exp(c, p=p):
                    """scores chunk c -> PSUM, then exp -> SBUF (f32r)."""
                    sc = scps.tile([128, sw], f32, tag="sc")
                    kc = kt_sb[:, c * 128:(c + 1) * 128]
                    for j in range(jn):
                        nc.tensor.matmul(
                            sc[:, j * nw:(j + 1) * nw], kc,
                            qt_sb[:, p * sw + j * nw: p * sw + (j + 1) * nw])
                    et = exp_pool.tile([128, sw], f32r, tag="et")
                    nc.scalar.activation(et[:], sc[:], FT.Exp,
                                         bias=alpha_sb[:, 2 * c:2 * c + 1],
                                         scale=SCALE)
                    return et

                # software pipeline: scores/exp run one chunk ahead of the
                # accumulating matmuls so the in-order PE never waits on ACT
                et_next = emit_scores_exp(0)
                for c in range(tch):
                    et = et_next
                    if c + 1 < tch:
                        et_next = emit_scores_exp(c + 1)
                    vc = v_sb[:, c * 128:(c + 1) * 128]
                    for j in range(jn):
                        ej = et[:, j * nw:(j + 1) * nw]
                        nc.tensor.matmul(acc_o[:, j * nw:(j + 1) * nw], vc, ej,
                                         start=(c == 0), stop=(c == tch - 1))
                        nc.tensor.matmul(acc_d[:, j * nw:(j + 1) * nw],
                                         ones_sb[:], ej,
                                         start=(c == 0), stop=(c == tch - 1))
                # normalize in [e, s] layout, add bv, transpose out
                # normalize + bias + transpose-out, sliced in 512-wide blocks
                # so the DVE chain pipelines with the PE transposes
                for b0 in range(0, sw, nw):
                    recip = epi.tile([128, nw], f32, tag="recip")
                    nc.vector.reciprocal_approx_fast(
                        recip[:], acc_d[:, b0:b0 + nw])
                    norm = epi.tile([128, nw], f32, tag="norm")
                    nc.vector.tensor_mul(norm[:], acc_o[:, b0:b0 + nw],
                                         recip[:])
                    norm2 = epi.tile([128, nw], f32, tag="norm2")
                    nc.vector.tensor_scalar_add(norm2[:], norm[:], bv_sb[:])
                    for k in range(nw // 128):
                        tp = scps.tile([128, 128], f32, tag="sc")
                        nc.tensor.transpose(
                            tp[:], norm2[:, k * 128:(k + 1) * 128],
                            ident_sb[:])
                        ot = outp.tile([128, 128], f32, tag="ot")
                        nc.vector.tensor_copy(ot[:], tp[:])
                        r0 = p * sw + b0 + k * 128
                        nc.sync.dma_start(out_d[r0: r0 + 128, :], ot[:])
    nc.compile()
    return nc


def make_in_maps(x, Wq, bq, Wk, Wv, bv, s=S, sq=SQ, n_cores=N_CORES):
    """Per-core input dict list. Core c -> batch c//(cores per batch),
    query slice (c % per_b) * sq."""
    x = np.asarray(x, np.float32)
    nb = x.shape[0]
    per_b = n_cores // nb
    wq_t = np.ascontiguousarray(np.asarray(Wq, np.float32).T)
    wk_t = np.ascontiguousarray(np.asarray(Wk, np.float32).T)
    wv_t = np.ascontiguousarray(np.asarray(Wv, np.float32).T)
    wtl = np.repeat((SCALE * (wk_t @ np.asarray(bq, np.float32)))
                    .reshape(D, 1), 2, axis=1)
    bvc = np.asarray(bv, np.float32).reshape(D, 1)
    maps = []
    for c in range(n_cores):
        b, h = c // per_b, c % per_b
        xt = np.ascontiguousarray(x[b].T)
        maps.append({
            "xT": xt,
            "xTq": np.ascontiguousarray(xt[:, h * sq:(h + 1) * sq]),
            "wqT": wq_t, "wkT": wk_t, "wvT": wv_t,
            "wtl": np.ascontiguousarray(wtl, dtype=np.float32),
            "bv": np.ascontiguousarray(bvc, dtype=np.float32),
        })
    return maps


_NC_CACHE = {}


def _get_nc():
    if "nc" not in _NC_CACHE:
        _NC_CACHE["nc"] = build_attention_bass()
    return _NC_CACHE["nc"]


def run_on_hw(inputs, trace=False, **kw):
    from concourse.bass_utils import run_bass_kernel_spmd
    nc = _get_nc()
    maps = make_in_maps(inputs["x"], inputs["Wq"], inputs["bq"], inputs["Wk"],
                        inputs["Wv"], inputs["bv"])
    res = run_bass_kernel_spmd(nc, maps, core_ids=list(range(N_CORES)),
                               trace=trace, **kw)
    nb = np.asarray(inputs["x"]).shape[0]
    per_b = N_CORES // nb
    out = np.empty((nb, S * D), np.float32)
    for c in range(N_CORES):
        b, h = c // per_b, c % per_b
        out[b, h * SQ * D:(h + 1) * SQ * D] = \
            np.asarray(res.results[c]["out"]).reshape(-1)
    return out, res


def kernel(**inputs):
    out, _ = run_on_hw(inputs, trace=False)
    return out
